# revision 3
# baseline (speedup 1.0000x reference)
"""BitwiseMLP Trainium2 kernel v10: 8-way data-parallel over the batch dim.

Math (per reference):
  h0 = x @ W0.T + b0; h0 = BN0(h0); s0 = sign(h0)
  h1 = s0 @ sign(W1).T + b1; h1 = BN1(h1); s1 = sign(h1)
  out = (s1 @ sign(W2).T + b2) * out_scale

v2 change vs baseline: L0 computes x@W0.T in TWO matmul passes instead of
three:
  main: xh @ Wh in f32r (xh, Wh rounded to EB-bit explicit mantissa, read
        exactly by the PE's float32r operand path), plus
  corr: ONE fp16 pass A @ B with A = (xl + alpha*x), B = (W + Wl/alpha):
        main + A@B = (1+alpha)*x@W + (1+1/alpha)*xl@Wl.
  With alpha = 2^-8 the parasitic xl@Wl term is ~2^-18 relative; the
  (1+alpha) factor folds into the BN scale. fp16 operands are pre-scaled
  by 2^+-6 (net 1) to stay in fp16's normal range. Simulated l2 rel err
  ~8.6e-3 at EB=11 (threshold 2e-2).
L1/L2 unchanged: exact +-1 fp8e4 matmuls with DoubleRow; fused BN+sign
(ScalarE) between layers; final eviction fused scale+bias.
"""
import os
import sys
import types

import numpy as np
import ml_dtypes

import concourse.bass as bass
import concourse.mybir as mybir
import concourse.tile as tile
from concourse import bacc
from concourse.bass_utils import run_bass_kernel_spmd


def _ensure_axon_hooks():
    """concourse.bass_utils imports antenv.axon_hooks when tracing is
    requested (BASS_TRACE=1). The trimmed image lacks that module, which
    would turn an optional profile into a crash — synthesize it, wiring the
    real NTFF hook when libaxon_pjrt.so is present."""
    try:
        import antenv.axon_hooks  # noqa: F401
        return
    except ImportError:
        pass
    try:
        import antenv
    except ImportError:
        return
    mod = types.ModuleType("antenv.axon_hooks")
    state = {"hook": None}
    mod.set_axon_ntff_profile_hook = lambda h: state.update(hook=h)
    mod.get_axon_ntff_profile_hook = lambda: state["hook"]
    sys.modules["antenv.axon_hooks"] = mod
    antenv.axon_hooks = mod
    so = "/opt/axon/libaxon_pjrt.so"
    if os.path.exists(so):
        try:
            from trn_agent_boot.trn_boot import _ntff_profile_via_ctypes
            mod.set_axon_ntff_profile_hook(_ntff_profile_via_ctypes(so))
            import concourse.bass_utils as _bu
            _real_upload = _bu.upload_artifacts

            def _safe_upload(tmpdir):
                try:
                    return _real_upload(tmpdir)
                except Exception:
                    return f"local:{tmpdir}"

            _bu.upload_artifacts = _safe_upload
        except Exception:
            pass


_ensure_axon_hooks()

dt = mybir.dt
P = 128
D = 1024
B = 65536
NCORES = 8
BS = B // NCORES          # 8192 batch rows per core
BT = 512                  # batch-tile width (columns of transposed activations)
NBT = BS // BT            # 16 batch tiles per core
KO = D // P               # 8 k-subtiles of 128 channels
EPS = 1e-5
EB = int(os.environ.get("V2_EB", "11"))   # explicit mantissa bits for f32r
ALPHA = 2.0 ** int(os.environ.get("V2_ALPHA", "-8"))
CSCALE = np.float32(64.0)  # fp16 balance scale (net 1 on the product)

LAST_RESULTS = None       # BassKernelResults of the most recent run
_NC = None                # cached compiled Bass module (build once per process)


def _round_sig(a: np.ndarray) -> np.ndarray:
    """Round fp32 magnitudes to EB explicit mantissa bits, half-to-even.
    Values of this form pass through the PE's float32r operand read
    exactly (verified on HW for EB=11)."""
    drop = np.uint64(23 - EB)
    u = a.view(np.uint32).astype(np.uint64)
    half = np.uint64(1) << (drop - np.uint64(1))
    one = np.uint64(1)
    r = (u + half - one + ((u >> drop) & one)) & ~((np.uint64(1) << drop) - one)
    return r.astype(np.uint32).view(np.float32)


def _build():
    nc = bacc.Bacc(num_devices=NCORES)
    xh = nc.dram_tensor("xh", [P, KO, BS], dt.float32r, kind="ExternalInput")
    xc = nc.dram_tensor("xc", [P, KO, BS], dt.float16, kind="ExternalInput")
    w0h = nc.dram_tensor("w0h", [P, KO, D], dt.float32r, kind="ExternalInput")
    w0c = nc.dram_tensor("w0c", [P, KO, D], dt.float16, kind="ExternalInput")
    w1 = nc.dram_tensor("w1", [P, KO, D], dt.float8e4, kind="ExternalInput")
    w2 = nc.dram_tensor("w2", [P, KO, D], dt.float8e4, kind="ExternalInput")
    vec = nc.dram_tensor("vec", [P, 6, KO], dt.float32, kind="ExternalInput")
    out = nc.dram_tensor("out", [P, KO, BS], dt.float32, kind="ExternalOutput")

    Sign = mybir.ActivationFunctionType.Sign
    Ident = mybir.ActivationFunctionType.Identity
    DR = mybir.MatmulPerfMode.DoubleRow
    ts = bass.ts

    with tile.TileContext(nc) as tc:
        with (
            tc.tile_pool(name="wpool", bufs=1) as wpool,
            tc.tile_pool(name="xpool", bufs=2) as xpool,
            tc.tile_pool(name="spool", bufs=2) as spool,
            tc.tile_pool(name="opool", bufs=3) as opool,
            tc.tile_pool(name="pspool", bufs=8, space="PSUM") as pspool,
        ):
            w0h_sb = wpool.tile([P, KO, D], dt.float32r)
            w0c_sb = wpool.tile([P, KO, D], dt.float16)
            w1_sb = wpool.tile([P, KO, D], dt.float8e4)
            w2_sb = wpool.tile([P, KO, D], dt.float8e4)
            vec_sb = wpool.tile([P, 6, KO], dt.float32)

            xh_t, xc_t = xh[:], xc[:]
            w0h_t, w0c_t = w0h[:], w0c[:]
            out_t = out[:]

            # bt0's x tiles first so the PE can start early; W0 chunks are
            # split per output-channel block m and follow in consumption order.
            sl0 = bass.ds(0, BT)
            xh_sb0 = xpool.tile([P, KO, BT], dt.float32r, tag="xh")
            xc_sb0 = xpool.tile([P, KO, BT], dt.float16, tag="xc")
            # deadline-ordered arrivals: f32r mains m0..m3 first, then the
            # correction tensors, then the rest.
            sl1 = bass.ds(BT, BT)
            xh_sb1 = xpool.tile([P, KO, BT], dt.float32r, tag="xh")
            xc_sb1 = xpool.tile([P, KO, BT], dt.float16, tag="xc")
            nc.sync.dma_start(xh_sb0[:, 0, :], xh_t[:, 0, sl0])
            nc.sync.dma_start(w0h_sb[:, :, ts(0, P)], w0h_t[:, :, ts(0, P)])
            for k in range(1, KO):
                nc.sync.dma_start(xh_sb0[:, k, :], xh_t[:, k, sl0])
            nc.sync.dma_start(w0h_sb[:, :, ts(1, P)], w0h_t[:, :, ts(1, P)])
            nc.sync.dma_start(w0h_sb[:, :, ts(2, P)], w0h_t[:, :, ts(2, P)])
            nc.sync.dma_start(w0h_sb[:, :, ts(3, P)], w0h_t[:, :, ts(3, P)])
            nc.sync.dma_start(xc_sb0, xc_t[:, :, sl0])
            nc.sync.dma_start(vec_sb, vec[:])
            nc.sync.dma_start(w0c_sb[:, :, ts(0, P)], w0c_t[:, :, ts(0, P)])
            for m in range(4, KO):
                msl = ts(m, P)
                nc.sync.dma_start(w0h_sb[:, :, msl], w0h_t[:, :, msl])
            for m in range(1, KO):
                msl = ts(m, P)
                nc.sync.dma_start(w0c_sb[:, :, msl], w0c_t[:, :, msl])
            nc.sync.dma_start(w1_sb, w1[:])
            nc.sync.dma_start(xh_sb1, xh_t[:, :, sl1])
            nc.sync.dma_start(xc_sb1, xc_t[:, :, sl1])
            nc.sync.dma_start(w2_sb, w2[:])

            for bt in range(NBT):
                sl = bass.ds(bt * BT, BT)
                if bt == 0:
                    xh_sb, xc_sb = xh_sb0, xc_sb0
                elif bt == 1:
                    xh_sb, xc_sb = xh_sb1, xc_sb1
                else:
                    xh_sb = xpool.tile([P, KO, BT], dt.float32r, tag="xh")
                    xc_sb = xpool.tile([P, KO, BT], dt.float16, tag="xc")
                    nc.sync.dma_start(xh_sb, xh_t[:, :, sl])
                    nc.sync.dma_start(xc_sb, xc_t[:, :, sl])

                # ---- L0: f32r main + merged fp16 correction, BN0+sign ----
                # signs live in 4 m-pair tiles so each DR k-pair matmul in
                # L1 depends only on the two sign blocks it reads, not on
                # the whole layer's activations.
                s0_p = [spool.tile([P, 2, BT], dt.float8e4, tag=f"s0_{i}",
                                   name=f"s0p{i}") for i in range(KO // 2)]
                for m in range(KO):
                    ps = pspool.tile([P, BT], dt.float32, tag="ps")
                    for k in range(KO):
                        nc.tensor.matmul(ps, w0h_sb[:, k, ts(m, P)],
                                         xh_sb[:, k, :],
                                         start=k == 0, stop=False)
                    for k in range(KO):
                        nc.tensor.matmul(ps, w0c_sb[:, k, ts(m, P)],
                                         xc_sb[:, k, :],
                                         start=False, stop=k == KO - 1)
                    nc.scalar.activation(s0_p[m // 2][:, m % 2, :], ps, Sign,
                                         bias=vec_sb[:, 1, m:m + 1],
                                         scale=vec_sb[:, 0, m:m + 1])

                # ---- L1: fp8 +-1 DoubleRow matmuls, fused BN1+sign ----
                s1_p = [spool.tile([P, 2, BT], dt.float8e4, tag=f"s1_{i}",
                                   name=f"s1p{i}") for i in range(KO // 2)]
                for m in range(KO):
                    ps = pspool.tile([P, BT], dt.float32, tag="ps")
                    for kp in range(KO // 2):
                        nc.tensor.matmul(ps, w1_sb[:, 2 * kp:2 * kp + 2, ts(m, P)],
                                         s0_p[kp][:, :, :],
                                         start=kp == 0, stop=kp == KO // 2 - 1,
                                         perf_mode=DR)
                    nc.scalar.activation(s1_p[m // 2][:, m % 2, :], ps, Sign,
                                         bias=vec_sb[:, 3, m:m + 1],
                                         scale=vec_sb[:, 2, m:m + 1])

                # ---- L2: fp8 +-1 DoubleRow matmuls, fused scale+bias ----
                for m in range(KO):
                    ps = pspool.tile([P, BT], dt.float32, tag="ps")
                    for kp in range(KO // 2):
                        nc.tensor.matmul(ps, w2_sb[:, 2 * kp:2 * kp + 2, ts(m, P)],
                                         s1_p[kp][:, :, :],
                                         start=kp == 0, stop=kp == KO // 2 - 1,
                                         perf_mode=DR)
                    o_sb = opool.tile([P, BT], dt.float32, tag="om")
                    nc.scalar.activation(o_sb, ps, Ident,
                                         bias=vec_sb[:, 5, m:m + 1],
                                         scale=vec_sb[:, 4, m:m + 1])
                    nc.sync.dma_start(out_t[:, m, sl], o_sb)

    nc.compile()
    return nc


def kernel(**inputs) -> np.ndarray:
    global LAST_RESULTS
    f32 = np.float32
    f16 = np.float16
    x = np.asarray(inputs["x"], f32)
    W0 = np.asarray(inputs["W0"], f32)
    b0 = np.asarray(inputs["b0"], f32)
    W1 = np.asarray(inputs["W1"], f32)
    b1 = np.asarray(inputs["b1"], f32)
    W2 = np.asarray(inputs["W2"], f32)
    b2 = np.asarray(inputs["b2"], f32)
    bn0_g = np.asarray(inputs["bn0_g"], f32)
    bn0_b = np.asarray(inputs["bn0_b"], f32)
    bn0_rm = np.asarray(inputs["bn0_rm"], f32)
    bn0_rv = np.asarray(inputs["bn0_rv"], f32)
    bn1_g = np.asarray(inputs["bn1_g"], f32)
    bn1_b = np.asarray(inputs["bn1_b"], f32)
    bn1_rm = np.asarray(inputs["bn1_rm"], f32)
    bn1_rv = np.asarray(inputs["bn1_rv"], f32)
    osc = np.asarray(inputs["out_scale"], f32)

    # per-channel affine folds (BN in eval mode); the matmul pair computes
    # (1+ALPHA)*(x@W0.T), so the L0 scale absorbs 1/(1+ALPHA):
    #   bn0(h+b0) = h*A0 + B0 ; bn1(h+b1) = h*A1 + B1 ; out = h*CS + CB
    ia = np.float32(1.0 / (1.0 + ALPHA))
    inv0 = (bn0_g / np.sqrt(bn0_rv + EPS)).astype(f32)
    inv1 = (bn1_g / np.sqrt(bn1_rv + EPS)).astype(f32)
    A0, B0 = (inv0 * ia).astype(f32), ((b0 - bn0_rm) * inv0 + bn0_b).astype(f32)
    A1, B1 = inv1, ((b1 - bn1_rm) * inv1 + bn1_b).astype(f32)
    CS, CB = osc, (b2 * osc).astype(f32)
    vec = np.stack([A0, B0, A1, B1, CS, CB])           # [6, D]
    vec_host = np.ascontiguousarray(
        vec.reshape(6, KO, P).transpose(2, 0, 1))      # [P, 6, KO]

    def pm(a):
        # [cols, D] -> partition-major [P, KO, cols]
        return np.ascontiguousarray(a.T.reshape(KO, P, -1).transpose(1, 0, 2))

    W0h = _round_sig(W0)
    w0h_host = pm(W0h)                                  # f32r main
    # merged correction stationary: (W0 + Wl/alpha) / CSCALE in fp16
    w0c_host = pm(((W0 + (W0 - W0h) / ALPHA) / CSCALE).astype(f16))
    e4m3 = mybir.dt.np(dt.float8e4)
    w1_host = pm(np.sign(W1).astype(e4m3))
    w2_host = pm(np.sign(W2).astype(e4m3))

    xh_full = _round_sig(x)
    xhT = pm(xh_full)                                   # [P, KO, B] f32r
    # merged correction moving: (xl + alpha*x) * CSCALE in fp16
    xcT = pm((((x - xh_full) + ALPHA * x) * CSCALE).astype(f16))

    shared = {
        "w0h": w0h_host, "w0c": w0c_host,
        "w1": w1_host, "w2": w2_host, "vec": vec_host,
    }
    in_maps = []
    for c in range(NCORES):
        bs = slice(c * BS, (c + 1) * BS)
        in_maps.append({
            **shared,
            "xh": np.ascontiguousarray(xhT[:, :, bs]),
            "xc": np.ascontiguousarray(xcT[:, :, bs]),
        })

    global _NC
    if _NC is None:
        _NC = _build()
    res = run_bass_kernel_spmd(_NC, in_maps, core_ids=list(range(NCORES)))
    LAST_RESULTS = res

    out = np.empty((B, D), f32)
    for c in range(NCORES):
        # [P, KO, BS] -> [BS, KO*P] with channel = ko*P + p
        o = res.results[c]["out"].transpose(2, 1, 0).reshape(BS, D)
        out[c * BS:(c + 1) * BS] = o
    return out
